# revision 1
# baseline (speedup 1.0000x reference)
"""Trainium2 Bass kernel for nn_AttentionWithCommunity.

Reference computation (see problem):
    in_active[c] = c in community_index
    comm = node2community[nodes]; use = in_active[comm]
    member_embedding[n] = sum_m score[comm[n],m] * E[neigh_com[comm[n],m]]
      where score is member_score masked by (m < member_num) & in_active[neigh],
      i.e. member_embedding[n] depends ONLY on comm[n] -> per-community agg.
    pred1 = MLP1([node_emb, E[nodes], member_embedding]); pred2 = MLP2(node_emb)
    out = where(use, pred1, pred2)

Strategy (8 cores, single SPMD launch, no collectives):
  Host does integer bookkeeping only: computes the active set, the valid
  (community, member) pairs, assigns each referenced active community to a
  core (greedy node-count balance), and co-locates every active node with
  its community's core.  All embedding-data movement and float math runs
  on device:
    Stage A: gather E rows for this core's valid pairs (indirect DMA,
             128 rows/op), matmul with a host-packed score matrix ->
             aggT[d, local_comm] (transposed agg), premultiplied by the
             member-slice of W1 into B3[local_comm, hidden].
    Stage B: for this core's active nodes: gather node_emb / E[nodes]
             rows, transpose via PE, build a one-hot(local comm) matrix
             on device and matmul against B3 (gathers the member-term
             WITHOUT a transpose), fused MLP1 -> pred1.
    Stage C: contiguous 1/8 shard of node_emb through MLP2 -> pred2.
  Host merges where(use, pred1, pred2).
"""

import os
import sys

import numpy as np

for _p in ("/opt/trn_rl_repo", "/root/.axon_site/_ro/trn_rl_repo"):
    if os.path.isdir(_p) and _p not in sys.path:
        sys.path.append(_p)

import concourse.bacc as bacc
import concourse.bass as bass
import concourse.mybir as mybir
from concourse.bass_utils import run_bass_kernel_spmd
from concourse.masks import make_identity
from concourse.tile import TileContext

N, V, C, M, D = 20000, 100000, 5000, 32, 128
NCORES = 8
P = 128
SUP = 4               # node tiles per matmul supertile (free dim 512)
NW = SUP * P

LAST_RESULTS = None   # set by kernel(); test harness reads exec_time_ns


def _roundup(x, m):
    return ((x + m - 1) // m) * m


def _build(NKA, NTB, NKC, NTC, NCOMMP, b2, b4):
    """Build the per-core SPMD Bass program. All sizes are compile-time."""
    f32 = mybir.dt.float32
    i32 = mybir.dt.int32
    NBP = NTB * P
    NCP = NTC * P

    nc = bacc.Bacc("TRN2", target_bir_lowering=False)
    E_h = nc.dram_tensor("E", [V, D], f32, kind="ExternalInput")
    ne_h = nc.dram_tensor("nodemb", [N, D], f32, kind="ExternalInput")
    cs_h = nc.dram_tensor("cshard", [NCP, D], f32, kind="ExternalInput")
    ap_h = nc.dram_tensor("a_pack", [P, NKA * NCOMMP], f32, kind="ExternalInput")
    pi_h = nc.dram_tensor("pair_idx", [P, NKA], i32, kind="ExternalInput")
    i1_h = nc.dram_tensor("idx1", [P, NTB], i32, kind="ExternalInput")
    i2_h = nc.dram_tensor("idx2", [P, NTB], i32, kind="ExternalInput")
    cf_h = nc.dram_tensor("commf", [1, NBP], f32, kind="ExternalInput")
    io_h = nc.dram_tensor("iota2", [P, NKC], f32, kind="ExternalInput")
    on_h = nc.dram_tensor("ones_row", [1, P], f32, kind="ExternalInput")
    w1ab_h = nc.dram_tensor("w1ab", [P, 2 * P], f32, kind="ExternalInput")
    w1c_h = nc.dram_tensor("w1c", [P, P], f32, kind="ExternalInput")
    w2_h = nc.dram_tensor("w2", [P, 1], f32, kind="ExternalInput")
    w3_h = nc.dram_tensor("w3", [P, 64], f32, kind="ExternalInput")
    w4_h = nc.dram_tensor("w4", [64, 1], f32, kind="ExternalInput")
    b1_h = nc.dram_tensor("b1c", [P, 1], f32, kind="ExternalInput")
    b3_h = nc.dram_tensor("b3c", [64, 1], f32, kind="ExternalInput")
    p1_h = nc.dram_tensor("pred1", [1, NBP], f32, kind="ExternalOutput")
    p2_h = nc.dram_tensor("pred2", [1, NCP], f32, kind="ExternalOutput")

    Relu = mybir.ActivationFunctionType.Relu
    Ident = mybir.ActivationFunctionType.Identity
    IOA = bass.IndirectOffsetOnAxis

    with TileContext(nc) as tc:
        with (
            tc.tile_pool(name="sbc", bufs=1) as sbc,
            tc.tile_pool(name="sbw", bufs=4) as sbw,
            tc.tile_pool(name="pst", bufs=2, space="PSUM") as pst,
            tc.tile_pool(name="psbc", bufs=2, space="PSUM") as psbc,
            tc.tile_pool(name="psh", bufs=2, space="PSUM") as psh,
            tc.tile_pool(name="psp", bufs=1, space="PSUM") as psp,
        ):
            # ---- constants ----
            ident = sbc.tile([P, P], f32)
            make_identity(nc, ident[:])
            pair_idx = sbc.tile([P, NKA], i32)
            nc.sync.dma_start(out=pair_idx[:], in_=pi_h[:])
            a_pack = sbc.tile([P, NKA * NCOMMP], f32)
            nc.sync.dma_start(out=a_pack[:], in_=ap_h[:])
            idx1 = sbc.tile([P, NTB], i32)
            nc.sync.dma_start(out=idx1[:], in_=i1_h[:])
            idx2 = sbc.tile([P, NTB], i32)
            nc.sync.dma_start(out=idx2[:], in_=i2_h[:])
            commf = sbc.tile([1, NBP], f32)
            nc.sync.dma_start(out=commf[:], in_=cf_h[:])
            iota2 = sbc.tile([P, NKC], f32)
            nc.sync.dma_start(out=iota2[:], in_=io_h[:])
            ones_row = sbc.tile([1, P], f32)
            nc.sync.dma_start(out=ones_row[:], in_=on_h[:])
            w1ab = sbc.tile([P, 2 * P], f32)
            nc.sync.dma_start(out=w1ab[:], in_=w1ab_h[:])
            w1c = sbc.tile([P, P], f32)
            nc.sync.dma_start(out=w1c[:], in_=w1c_h[:])
            w2 = sbc.tile([P, 1], f32)
            nc.sync.dma_start(out=w2[:], in_=w2_h[:])
            w3 = sbc.tile([P, 64], f32)
            nc.sync.dma_start(out=w3[:], in_=w3_h[:])
            w4 = sbc.tile([64, 1], f32)
            nc.sync.dma_start(out=w4[:], in_=w4_h[:])
            b1c = sbc.tile([P, 1], f32)
            nc.sync.dma_start(out=b1c[:], in_=b1_h[:])
            b3c = sbc.tile([64, 1], f32)
            nc.sync.dma_start(out=b3c[:], in_=b3_h[:])
            pred1 = sbc.tile([1, NBP], f32)
            pred2 = sbc.tile([1, NCP], f32)

            # ---- stage A: aggT[d, lc] = sum_pairs E[neigh,d]*score ----
            aggT = sbc.tile([P, NCOMMP], f32)
            with tc.tile_pool(name="psa", bufs=1, space="PSUM") as psa:
                aggT_ps = psa.tile([P, NCOMMP], f32, space="PSUM")
                for kt in range(NKA):
                    g = sbw.tile([P, P], f32, tag="ga")
                    nc.gpsimd.indirect_dma_start(
                        out=g[:], out_offset=None, in_=E_h[:],
                        in_offset=IOA(ap=pair_idx[:, kt:kt + 1], axis=0))
                    nc.tensor.matmul(
                        out=aggT_ps[:], lhsT=g[:],
                        rhs=a_pack[:, kt * NCOMMP:(kt + 1) * NCOMMP],
                        start=(kt == 0), stop=(kt == NKA - 1))
                nc.vector.tensor_copy(out=aggT[:], in_=aggT_ps[:])

            # B3[lc, h] = (agg @ W1c)[lc, h], per 128-community block
            B3 = sbc.tile([P, NKC * P], f32)
            for j in range(NKC):
                b3p = pst.tile([P, P], f32, space="PSUM", tag="tp")
                nc.tensor.matmul(out=b3p[:], lhsT=aggT[:, j * P:(j + 1) * P],
                                 rhs=w1c[:], start=True, stop=True)
                nc.vector.tensor_copy(out=B3[:, j * P:(j + 1) * P], in_=b3p[:])

            # ---- stage B: active nodes, groups of SUP node-tiles ----
            for gidx in range(NTB // SUP):
                xT = sbw.tile([P, NW], f32, tag="xT")
                yT = sbw.tile([P, NW], f32, tag="yT")
                for j in range(SUP):
                    t = gidx * SUP + j
                    x1 = sbw.tile([P, P], f32, tag="g1")
                    nc.gpsimd.indirect_dma_start(
                        out=x1[:], out_offset=None, in_=ne_h[:],
                        in_offset=IOA(ap=idx1[:, t:t + 1], axis=0))
                    tp1 = pst.tile([P, P], f32, space="PSUM", tag="tp")
                    nc.tensor.transpose(out=tp1[:], in_=x1[:], identity=ident[:])
                    nc.vector.tensor_copy(out=xT[:, j * P:(j + 1) * P], in_=tp1[:])
                    x2 = sbw.tile([P, P], f32, tag="g2")
                    nc.gpsimd.indirect_dma_start(
                        out=x2[:], out_offset=None, in_=E_h[:],
                        in_offset=IOA(ap=idx2[:, t:t + 1], axis=0))
                    tp2 = pst.tile([P, P], f32, space="PSUM", tag="tp")
                    nc.tensor.transpose(out=tp2[:], in_=x2[:], identity=ident[:])
                    nc.vector.tensor_copy(out=yT[:, j * P:(j + 1) * P], in_=tp2[:])

                bc = psbc.tile([P, NW], f32, space="PSUM", tag="bc")
                nc.tensor.matmul(out=bc[:], lhsT=ones_row[:],
                                 rhs=commf[0:1, gidx * NW:(gidx + 1) * NW],
                                 start=True, stop=True)
                HT = psh.tile([P, NW], f32, space="PSUM", tag="ht")
                nc.tensor.matmul(out=HT[:], lhsT=w1ab[:, 0:P], rhs=xT[:],
                                 start=True, stop=False)
                nc.tensor.matmul(out=HT[:], lhsT=w1ab[:, P:2 * P], rhs=yT[:],
                                 start=False, stop=False)
                for jc in range(NKC):
                    oh = sbw.tile([P, NW], f32, tag="oh")
                    nc.vector.tensor_tensor(
                        out=oh[:], in0=bc[:],
                        in1=iota2[:, jc:jc + 1].to_broadcast([P, NW]),
                        op=mybir.AluOpType.is_equal)
                    nc.tensor.matmul(out=HT[:], lhsT=B3[:, jc * P:(jc + 1) * P],
                                     rhs=oh[:], start=False, stop=(jc == NKC - 1))
                HTs = sbw.tile([P, NW], f32, tag="hts")
                nc.scalar.activation(out=HTs[:], in_=HT[:], func=Relu,
                                     bias=b1c[:, :1], scale=1.0)
                p1p = psp.tile([1, NW], f32, space="PSUM", tag="pp")
                nc.tensor.matmul(out=p1p[:], lhsT=w2[:], rhs=HTs[:],
                                 start=True, stop=True)
                nc.scalar.activation(
                    out=pred1[0:1, gidx * NW:(gidx + 1) * NW], in_=p1p[:],
                    func=Ident, bias=float(b2), scale=1.0)

            # ---- stage C: contiguous node shard through MLP2 ----
            for gidx in range(NTC // SUP):
                zT = sbw.tile([P, NW], f32, tag="zT")
                for j in range(SUP):
                    t = gidx * SUP + j
                    xc = sbw.tile([P, P], f32, tag="gc")
                    nc.sync.dma_start(out=xc[:], in_=cs_h[t * P:(t + 1) * P, :])
                    tpc = pst.tile([P, P], f32, space="PSUM", tag="tp")
                    nc.tensor.transpose(out=tpc[:], in_=xc[:], identity=ident[:])
                    nc.vector.tensor_copy(out=zT[:, j * P:(j + 1) * P], in_=tpc[:])
                H2 = psh.tile([64, NW], f32, space="PSUM", tag="ht")
                nc.tensor.matmul(out=H2[:], lhsT=w3[:], rhs=zT[:],
                                 start=True, stop=True)
                H2s = sbw.tile([64, NW], f32, tag="h2s")
                nc.scalar.activation(out=H2s[:], in_=H2[:], func=Relu,
                                     bias=b3c[:, :1], scale=1.0)
                p2p = psp.tile([1, NW], f32, space="PSUM", tag="pp")
                nc.tensor.matmul(out=p2p[:], lhsT=w4[:], rhs=H2s[:],
                                 start=True, stop=True)
                nc.scalar.activation(
                    out=pred2[0:1, gidx * NW:(gidx + 1) * NW], in_=p2p[:],
                    func=Ident, bias=float(b4), scale=1.0)

            nc.sync.dma_start(out=p1_h[:], in_=pred1[:])
            nc.sync.dma_start(out=p2_h[:], in_=pred2[:])
    nc.compile()
    return nc


def kernel(node_emb, member_score, community_embeddings, W1, b1, W2, b2,
           W3, b3, W4, b4, node2community, community2node, member_num,
           community_index, nodes):
    global LAST_RESULTS

    node_emb = np.ascontiguousarray(np.asarray(node_emb, np.float32))
    member_score = np.asarray(member_score, np.float32)
    E = np.ascontiguousarray(np.asarray(community_embeddings, np.float32))
    W1 = np.asarray(W1, np.float32)
    b1 = np.asarray(b1, np.float32)
    W2 = np.asarray(W2, np.float32)
    b2 = np.asarray(b2, np.float32)
    W3 = np.asarray(W3, np.float32)
    b3 = np.asarray(b3, np.float32)
    W4 = np.asarray(W4, np.float32)
    b4 = np.asarray(b4, np.float32)
    node2community = np.asarray(node2community).astype(np.int64)
    community2node = np.asarray(community2node).astype(np.int64)
    member_num = np.asarray(member_num).astype(np.int64)
    community_index = np.asarray(community_index).astype(np.int64)
    nodes = np.asarray(nodes).astype(np.int64)

    # ---------- host index bookkeeping ----------
    in_active = np.zeros(C, bool)
    in_active[community_index] = True
    comm = node2community[nodes]                    # [N] community per node
    use = in_active[comm]                           # [N]
    neigh_com = node2community[community2node]      # [C, M], values < C
    len_mask = np.arange(M)[None, :] < member_num[:, None]
    valid = len_mask & in_active[neigh_com]         # [C, M]
    score = np.where(valid, member_score, 0.0).astype(np.float32)

    active_ids = np.nonzero(use)[0]
    ref_comms = np.unique(comm[active_ids]) if len(active_ids) else np.empty(0, np.int64)

    node_cnt = np.zeros(C, np.int64)
    if len(active_ids):
        np.add.at(node_cnt, comm[active_ids], 1)
    pair_cnt = valid.sum(1)

    # greedy: assign communities to cores balancing node count, then pairs
    comm_core = np.full(C, -1, np.int32)
    core_comms = [[] for _ in range(NCORES)]
    cn = np.zeros(NCORES, np.int64)
    cp = np.zeros(NCORES, np.int64)
    for c in ref_comms[np.argsort(-node_cnt[ref_comms], kind="stable")]:
        k = int(np.lexsort((cp, cn))[0])
        comm_core[c] = k
        core_comms[k].append(int(c))
        cn[k] += node_cnt[c]
        cp[k] += pair_cnt[c]

    core_nodes = [active_ids[comm_core[comm[active_ids]] == k]
                  for k in range(NCORES)]
    core_pairs = []
    for k in range(NCORES):
        cs = np.asarray(core_comms[k], np.int64)
        if len(cs):
            lcs, ms = np.nonzero(valid[cs])     # local comm idx, member idx
            core_pairs.append((neigh_com[cs[lcs], ms].astype(np.int64),
                               score[cs[lcs], ms], lcs.astype(np.int64)))
        else:
            core_pairs.append((np.empty(0, np.int64), np.empty(0, np.float32),
                               np.empty(0, np.int64)))

    NCOMMP = max(_roundup(max((len(c) for c in core_comms), default=0), P), P)
    NKC = NCOMMP // P
    NKA = max(_roundup(max(len(p[0]) for p in core_pairs), P) // P, 1)
    NTB = max(_roundup(max((len(n) for n in core_nodes), default=0), NW) // P, SUP)
    NBP = NTB * P
    NSH = N // NCORES                               # contiguous shard rows
    NTC = _roundup(NSH, NW) // P
    NCP = NTC * P

    nc = _build(NKA, NTB, NKC, NTC, NCOMMP, float(b2[0]), float(b4[0]))

    # ---------- per-core input packing ----------
    w1ab = np.ascontiguousarray(np.concatenate([W1[0:P], W1[P:2 * P]], axis=1))
    w1c = np.ascontiguousarray(W1[2 * P:3 * P])
    iota2 = (np.arange(P, dtype=np.float32)[:, None]
             + P * np.arange(NKC, dtype=np.float32)[None, :])
    iota2 = np.ascontiguousarray(iota2)
    ones_row = np.ones((1, P), np.float32)
    b1c = np.ascontiguousarray(b1[:, None])
    b3c = np.ascontiguousarray(b3[:, None])

    in_maps = []
    for k in range(NCORES):
        neigh, sc, lcs = core_pairs[k]
        npair = len(neigh)
        a_pack = np.zeros((P, NKA, NCOMMP), np.float32)
        pair_idx = np.zeros((P, NKA), np.int32)
        pp = np.arange(npair)
        a_pack[pp % P, pp // P, lcs] = sc
        pair_idx[pp % P, pp // P] = neigh.astype(np.int32)

        ids = core_nodes[k]
        nb = len(ids)
        idx1 = np.zeros((P, NTB), np.int32)
        idx2 = np.zeros((P, NTB), np.int32)
        commf = np.zeros((1, NBP), np.float32)
        ss = np.arange(nb)
        idx1[ss % P, ss // P] = ids.astype(np.int32)
        idx2[ss % P, ss // P] = nodes[ids].astype(np.int32)
        # local community index of each active node, in slot order
        lc_of = np.zeros(C, np.int64)
        cs = np.asarray(core_comms[k], np.int64)
        if len(cs):
            lc_of[cs] = np.arange(len(cs))
        commf[0, :nb] = lc_of[comm[ids]].astype(np.float32)

        cshard = np.zeros((NCP, D), np.float32)
        cshard[:NSH] = node_emb[k * NSH:(k + 1) * NSH]

        in_maps.append(dict(
            E=E, nodemb=node_emb, cshard=cshard,
            a_pack=np.ascontiguousarray(a_pack.reshape(P, NKA * NCOMMP)),
            pair_idx=pair_idx, idx1=idx1, idx2=idx2, commf=commf,
            iota2=iota2, ones_row=ones_row, w1ab=w1ab, w1c=w1c,
            w2=W2, w3=W3, w4=W4, b1c=b1c, b3c=b3c))

    res = run_bass_kernel_spmd(nc, in_maps, core_ids=list(range(NCORES)))
    LAST_RESULTS = res

    out = np.empty(N, np.float32)
    for k in range(NCORES):
        out[k * NSH:(k + 1) * NSH] = res.results[k]["pred2"][0, :NSH]
    for k in range(NCORES):
        ids = core_nodes[k]
        if len(ids):
            out[ids] = res.results[k]["pred1"][0, :len(ids)]
    return out
